# revision 10
# baseline (speedup 1.0000x reference)
"""Batched GAT kernel for Trainium2 (Bass/Tile), data-parallel over batch on 8 cores.

Math (per graph b, head h):
    hfeat = x @ W; e_src/e_dst per head; l = lrelu(e_dst[i]+e_src[j], 0.2)
    att = softmax_j(mask ? l : -inf); out = att @ hfeat + bias.

Key structure:
  - Softmax is scale-invariant per column i: divide P = exp(lrelu(l)) by
    exp(e_dst[i]).  With v1=exp(e_src), v2=exp(0.2 e_src), w=exp(-0.8 e_dst):
        P[j,i] = max(v1[j], w[i]*v2[j])
    so no exp over the N^2 grid: one dual-scalar tensor_scalar per tile
    (mult+max with per-partition scalar columns) + one mask multiply, in bf16.
  - Mask: exact fp32 compare in natural layout -> bf16 {0,1} -> batched
    [128,1024] DMA xbar transposes (contiguous [128,it,jt,128] dsts).
    ALL xbar transposes ride the sync HWDGE ring: concurrent transposes on
    both rings corrupt data on HW.
  - x^T via bf16 xbar transposes (x is cast during the SWDGE load).
  - Aggregation matmul in bf16: lhsT = [hfeat | ones] -> psum rows 0..63 =
    out^T, row 64 = softmax denominator.  PE retransposes out^T; ScalarE
    applies the reciprocal scale; V only does reciprocal + bias.
  - Manual 2-stage software pipeline: graph b+1's precompute is emitted
    between graph b's first and second head so its DMA/PE work overlaps
    graph b's head compute (engine queues are FIFO in program order).
"""

import sys

if "/opt/trn_rl_repo" not in sys.path:
    sys.path.insert(0, "/opt/trn_rl_repo")

import numpy as np

B, N, D, H, F = 16, 1024, 256, 4, 64
N_CORES = 8
B_LOCAL = B // N_CORES

_CACHE = {}


def _build(b_local, n, d, h_heads, f_dim):
    from contextlib import ExitStack

    import concourse.bass as bass  # noqa: F401
    import concourse.tile as tile
    from concourse import bacc, mybir
    from concourse.bass import ts
    from concourse.masks import make_identity

    fp32 = mybir.dt.float32
    bf16 = mybir.dt.bfloat16
    AF = mybir.ActivationFunctionType
    OP = mybir.AluOpType

    HF = h_heads * f_dim
    NT = n // 128
    DK = d // 128
    KK = HF // 128
    F1 = f_dim + 1
    halves = [(s, min(s + 512, n)) for s in range(0, n, 512)]

    nc = bacc.Bacc(None, target_bir_lowering=False)
    x_d = nc.dram_tensor("x", [b_local, n, d], fp32, kind="ExternalInput")
    adj_d = nc.dram_tensor("adj", [b_local, n, n], fp32, kind="ExternalInput")
    w_d = nc.dram_tensor("W", [d, HF], fp32, kind="ExternalInput")
    asrc_d = nc.dram_tensor("a_src", [h_heads, f_dim], fp32, kind="ExternalInput")
    adst_d = nc.dram_tensor("a_dst", [h_heads, f_dim], fp32, kind="ExternalInput")
    bias_d = nc.dram_tensor("bias", [HF], fp32, kind="ExternalInput")
    out_d = nc.dram_tensor("out", [b_local, n, HF], fp32, kind="ExternalOutput")

    with ExitStack() as ctx:
        tc = ctx.enter_context(tile.TileContext(nc))
        const = ctx.enter_context(tc.tile_pool(name="const", bufs=1))
        io = ctx.enter_context(tc.tile_pool(name="io", bufs=2))
        work = ctx.enter_context(tc.tile_pool(name="work", bufs=2))
        ppool = ctx.enter_context(tc.tile_pool(name="ppool", bufs=6))
        rpool = ctx.enter_context(tc.tile_pool(name="rpool", bufs=4))
        dram = ctx.enter_context(tc.tile_pool(name="dram", bufs=2, space="DRAM"))
        psum_agg = ctx.enter_context(
            tc.tile_pool(name="psum_agg", bufs=2, space="PSUM")
        )
        psum_tp = ctx.enter_context(tc.tile_pool(name="psum_tp", bufs=4, space="PSUM"))

        # ---- constants ----
        ident = const.tile([128, 128], fp32, name="ident")
        make_identity(nc, ident)

        bias_bc = const.tile([128, HF], fp32, name="bias_bc")
        nc.scalar.dma_start(out=bias_bc, in_=bias_d[:].partition_broadcast(128))

        w_sb = const.tile([128, DK, HF], fp32, name="w_sb")
        nc.scalar.dma_start(out=w_sb, in_=w_d[:].rearrange("(k p) m -> p k m", p=128))
        w_bf = const.tile([128, DK, HF], bf16, name="w_bf")
        nc.vector.tensor_copy(w_bf, w_sb)

        # W^T via PE transposes (to project a_src/a_dst back to input dim)
        wt_sb = const.tile([128, KK, d], fp32, name="wt_sb")
        for dk in range(DK):
            for kk in range(KK):
                tp = psum_tp.tile([128, 512], fp32, name="tp", tag="tp")
                nc.tensor.transpose(tp[:, 0:128], w_sb[:, dk, ts(kk, 128)], ident)
                nc.vector.tensor_copy(wt_sb[:, kk, ts(dk, 128)], tp[:, 0:128])

        # Block-diagonal attention vectors: A[hf, h'] = a_vec[h, f] iff h' == h
        a_tiles = {}
        for nm, src in (("asrc", asrc_d), ("adst", adst_d)):
            a_sb = const.tile([128, KK, h_heads], fp32, name=f"a_{nm}")
            nc.vector.memset(a_sb, 0.0)
            for hh in range(h_heads):
                kk = (hh * f_dim) // 128
                r0 = hh * f_dim - kk * 128
                nc.gpsimd.dma_start(out=a_sb[r0:r0 + f_dim, kk, hh], in_=src[hh, :])
            a_tiles[nm] = a_sb

        # w_vec[d, h] = sum_hf W^T[hf, d] * A[hf, h]  (so e = x @ w_vec)
        wv_bf = {}
        for nm in ("asrc", "adst"):
            wv_sb = const.tile([128, DK, h_heads], fp32, name=f"wv_{nm}")
            for dk in range(DK):
                tp = psum_tp.tile([128, 512], fp32, name="tp", tag="tp")
                for kk in range(KK):
                    nc.tensor.matmul(
                        tp[:, 0:h_heads],
                        wt_sb[:, kk, ts(dk, 128)],
                        a_tiles[nm][:, kk, :],
                        start=(kk == 0),
                        stop=(kk == KK - 1),
                    )
                nc.vector.tensor_copy(wv_sb[:, dk, :], tp[:, 0:h_heads])
            wvb = const.tile([128, DK, h_heads], bf16, name=f"wvb_{nm}")
            nc.vector.tensor_copy(wvb, wv_sb)
            wv_bf[nm] = wvb

        def pre(b):
            """Per-graph precompute: x^T, masks, features, e-derived tiles."""
            # adj loads first on the sync ring: no deps, stream immediately
            adj_tiles = []
            for it in range(NT):
                adj_sb = io.tile([128, n], fp32, name="adj_sb", tag="adj", bufs=4)
                nc.sync.dma_start(out=adj_sb, in_=adj_d[b][ts(it, 128), :])
                adj_tiles.append(adj_sb)

            # x loads directly as bf16 (SWDGE cast DMA)
            x_bf = io.tile([128, NT, d], bf16, name="x_bf", tag="xbf")
            nc.gpsimd.dma_start(
                out=x_bf, in_=x_d[b].rearrange("(t p) c -> p t c", p=128)
            )

            # x^T (bf16) via DMA xbar (contiguous [128, nt, dk, 128] dsts)
            xt_bf = io.tile([128, NT, DK, 128], bf16, name="xt_bf", tag="xt")
            for nt in range(NT):
                nc.sync.dma_start_transpose(xt_bf[:, nt, :, :], x_bf[:, nt, :])

            # mask compare + transposes
            m01 = io.tile([128, NT, NT, 128], bf16, name="m01", tag="m01")
            for it in range(NT):
                mnat = io.tile([128, n], bf16, name="mnat", tag="mnat")
                nc.vector.tensor_scalar(mnat, adj_tiles[it], 0.5, None, op0=OP.is_gt)
                nc.sync.dma_start_transpose(m01[:, it, :, :], mnat)

            # e rows first (they gate the wb broadcast + v-columns chain)
            e_sb = {}
            for nm in ("asrc", "adst"):
                esb = work.tile([h_heads, n], fp32, name="esb", tag=f"e_{nm}")
                for s, e in halves:
                    tpe = psum_tp.tile([128, 512], fp32, name="tpe", tag="tp")
                    nt0 = s // 128
                    for dk in range(DK):
                        nc.tensor.matmul(
                            tpe[0:h_heads, 0:e - s],
                            wv_bf[nm][:, dk, :],
                            xt_bf[:, nt0:nt0 + 4, dk, :],
                            start=(dk == 0),
                            stop=(dk == DK - 1),
                        )
                    nc.vector.tensor_copy(esb[:, s:e], tpe[0:h_heads, 0:e - s])
                e_sb[nm] = esb

            # v1 = exp(e_src), v2 = exp(0.2 e_src) (bf16 rows, xbar-transposable);
            # w = exp(-0.8 e_dst) (bf16 row for the broadcast)
            v1row = work.tile([16, n], bf16, name="v1row", tag="v1r")
            nc.scalar.activation(v1row[0:h_heads, :], e_sb["asrc"], AF.Exp)
            v2row = work.tile([16, n], bf16, name="v2row", tag="v2r")
            nc.scalar.activation(v2row[0:h_heads, :], e_sb["asrc"], AF.Exp, scale=0.2)
            wrow = work.tile([h_heads, n], bf16, name="wrow", tag="wr")
            nc.scalar.activation(wrow, e_sb["adst"], AF.Exp, scale=-0.8)

            # w rows -> DRAM -> per-head partition broadcast (bf16)
            wd = dram.tile([h_heads, n], bf16, name="wd", tag="wd")
            nc.scalar.dma_start(out=wd, in_=wrow)
            wb = io.tile([128, h_heads, n], bf16, name="wb", tag="wb")
            for hh in range(h_heads):
                nc.scalar.dma_start(
                    out=wb[:, hh, :], in_=wd[hh].partition_broadcast(128)
                )

            # v1/v2 -> per-partition columns via one xbar transpose each
            # (TS scalar operands must be fp32, so upcast the [128,NT,H] slice)
            vcs = {}
            for vrow, tag in ((v1row, "v1c"), (v2row, "v2c")):
                vt = work.tile([128, NT, 16], bf16, name=f"t{tag}", tag=f"t{tag}")
                nc.sync.dma_start_transpose(vt, vrow)
                vc = io.tile([128, NT, h_heads], fp32, name=f"c{tag}", tag=tag)
                nc.vector.tensor_copy(vc, vt[:, :, 0:h_heads])
                vcs[tag] = vc

            # features: h = x @ W -> [j, hh, ff] bf16 (+ ones col)
            ha = io.tile([128, NT, h_heads, F1], bf16, name="ha", tag="haug")
            nc.gpsimd.memset(ha[:, :, :, f_dim:F1], 1.0)
            for nt in range(NT):
                tp = psum_tp.tile([128, 512], fp32, name="tp", tag="tp")
                for dk in range(DK):
                    nc.tensor.matmul(
                        tp[:, 0:HF],
                        xt_bf[:, nt, dk, :],
                        w_bf[:, dk, :],
                        start=(dk == 0),
                        stop=(dk == DK - 1),
                    )
                nc.scalar.copy(
                    ha[:, nt, :, 0:f_dim],
                    tp[:, 0:HF].rearrange("p (hh ff) -> p hh ff", hh=h_heads),
                )

            ostage = io.tile([128, NT, HF], fp32, name="ostage", tag="ostage")
            return {
                "ha": ha, "m01": m01, "wb": wb,
                "v1c": vcs["v1c"], "v2c": vcs["v2c"], "ostage": ostage,
            }

        def head(b, t, hh):
            """One head: P tiles, masked, aggregated, normalized into ostage."""
            agg = psum_agg.tile([F1, n], fp32, name="agg", tag="agg")
            for jt in range(NT):
                # P = max(w[i]*v2[j], v1[j]); Pm = P * mask  (bf16)
                pm = ppool.tile([128, n], bf16, name="pm", tag="pm")
                nc.vector.tensor_scalar(
                    pm,
                    t["wb"][:, hh, :],
                    t["v2c"][:, jt, hh:hh + 1],
                    t["v1c"][:, jt, hh:hh + 1],
                    op0=OP.mult,
                    op1=OP.max,
                )
                eng = nc.gpsimd if (jt % 8 in (2, 5, 7)) else nc.vector
                eng.tensor_tensor(pm, pm, t["m01"][:, :, jt, :], op=OP.mult)
                for s, e in halves:
                    nc.tensor.matmul(
                        agg[:, s:e],
                        t["ha"][:, jt, hh, :],
                        pm[:, s:e],
                        start=(jt == 0),
                        stop=(jt == NT - 1),
                    )

            # finalize: psum rows [0..F) = out^T, row F = denominator
            agg_sb = work.tile([F1, n], fp32, name="agg_sb", tag="aggsb")
            nc.scalar.copy(agg_sb, agg)
            for c in range(NT):
                tp = psum_tp.tile([128, 512], fp32, name="tp", tag="tp")
                nc.tensor.transpose(
                    tp[:, 0:F1], agg_sb[:, ts(c, 128)], ident[0:F1, 0:F1]
                )
                rcp = rpool.tile([128, 1], fp32, name="rcp", tag="rcp")
                nc.vector.reciprocal(rcp, tp[:, f_dim:F1])
                nc.scalar.activation(
                    t["ostage"][:, c, hh * f_dim:(hh + 1) * f_dim],
                    tp[:, 0:f_dim],
                    AF.Copy,
                    bias=0.0,
                    scale=rcp,
                )

        def finish(b, t):
            for c in range(NT):
                nc.vector.tensor_tensor(
                    t["ostage"][:, c, :], t["ostage"][:, c, :], bias_bc, op=OP.add
                )
            nc.scalar.dma_start(
                out=out_d[b].rearrange("(t p) m -> p t m", p=128), in_=t["ostage"]
            )

        # 2-stage software pipeline: pre(b+1) is emitted after the first head
        # of graph b so its DMA/PE work overlaps graph b's head compute.
        tiles = pre(0)
        for b in range(b_local):
            head(b, tiles, 0)
            nxt = pre(b + 1) if b + 1 < b_local else None
            for hh in range(1, h_heads):
                head(b, tiles, hh)
            finish(b, tiles)
            tiles = nxt

    nc.finalize()
    return nc


def _get_nc(shape_key):
    if shape_key not in _CACHE:
        _CACHE[shape_key] = _build(*shape_key)
    return _CACHE[shape_key]


def kernel(x, adj, W, a_src, a_dst, bias):
    from concourse.bass_utils import run_bass_kernel_spmd

    x = np.ascontiguousarray(x, dtype=np.float32)
    adj = np.ascontiguousarray(adj, dtype=np.float32)
    W = np.ascontiguousarray(W, dtype=np.float32)
    a_src = np.ascontiguousarray(a_src, dtype=np.float32)
    a_dst = np.ascontiguousarray(a_dst, dtype=np.float32)
    bias = np.ascontiguousarray(bias, dtype=np.float32)

    nc = _get_nc((B_LOCAL, N, D, H, F))
    in_maps = []
    for c in range(N_CORES):
        sl = slice(c * B_LOCAL, (c + 1) * B_LOCAL)
        in_maps.append(
            {
                "x": x[sl],
                "adj": adj[sl],
                "W": W,
                "a_src": a_src,
                "a_dst": a_dst,
                "bias": bias,
            }
        )
    res = run_bass_kernel_spmd(nc, in_maps, core_ids=list(range(N_CORES)))
    return np.concatenate([r["out"] for r in res.results], axis=0)


# revision 11
# speedup vs baseline: 1.0226x; 1.0226x over previous
"""Batched GAT kernel for Trainium2 (Bass/Tile), data-parallel over batch on 8 cores.

Math (per graph b, head h):
    hfeat = x @ W; e_src/e_dst per head; l = lrelu(e_dst[i]+e_src[j], 0.2)
    att = softmax_j(mask ? l : -inf); out = att @ hfeat + bias.

Key structure:
  - Softmax is scale-invariant per column i: divide P = exp(lrelu(l)) by
    exp(e_dst[i]).  With v1=exp(e_src), v2=exp(0.2 e_src), w=exp(-0.8 e_dst):
        P[j,i] = max(v1[j], w[i]*v2[j])
    so no exp over the N^2 grid: one dual-scalar tensor_scalar per tile
    (mult+max with per-partition scalar columns) + one mask multiply, in bf16.
  - Mask: exact fp32 compare in natural layout -> bf16 {0,1} -> batched
    [128,1024] DMA xbar transposes (contiguous [128,it,jt,128] dsts).
    ALL xbar transposes ride the sync HWDGE ring: concurrent transposes on
    both rings corrupt data on HW.
  - x^T via bf16 xbar transposes (x is cast during the SWDGE load).
  - Aggregation matmul in bf16: lhsT = [hfeat | ones] -> psum rows 0..63 =
    out^T, row 64 = softmax denominator.  PE retransposes out^T; ScalarE
    applies the reciprocal scale; V only does reciprocal + bias.
  - Manual 2-stage software pipeline: graph b+1's precompute is emitted
    between graph b's first and second head so its DMA/PE work overlaps
    graph b's head compute (engine queues are FIFO in program order).
"""

import sys

if "/opt/trn_rl_repo" not in sys.path:
    sys.path.insert(0, "/opt/trn_rl_repo")

import numpy as np

B, N, D, H, F = 16, 1024, 256, 4, 64
N_CORES = 8
B_LOCAL = B // N_CORES

_CACHE = {}


def _build(b_local, n, d, h_heads, f_dim):
    from contextlib import ExitStack

    import concourse.bass as bass  # noqa: F401
    import concourse.tile as tile
    from concourse import bacc, mybir
    from concourse.bass import ts
    from concourse.masks import make_identity

    fp32 = mybir.dt.float32
    bf16 = mybir.dt.bfloat16
    AF = mybir.ActivationFunctionType
    OP = mybir.AluOpType

    HF = h_heads * f_dim
    NT = n // 128
    DK = d // 128
    KK = HF // 128
    F1 = f_dim + 1
    halves = [(s, min(s + 512, n)) for s in range(0, n, 512)]

    nc = bacc.Bacc(None, target_bir_lowering=False)
    x_d = nc.dram_tensor("x", [b_local, n, d], fp32, kind="ExternalInput")
    adj_d = nc.dram_tensor("adj", [b_local, n, n], fp32, kind="ExternalInput")
    w_d = nc.dram_tensor("W", [d, HF], fp32, kind="ExternalInput")
    asrc_d = nc.dram_tensor("a_src", [h_heads, f_dim], fp32, kind="ExternalInput")
    adst_d = nc.dram_tensor("a_dst", [h_heads, f_dim], fp32, kind="ExternalInput")
    bias_d = nc.dram_tensor("bias", [HF], fp32, kind="ExternalInput")
    out_d = nc.dram_tensor("out", [b_local, n, HF], fp32, kind="ExternalOutput")

    with ExitStack() as ctx:
        tc = ctx.enter_context(tile.TileContext(nc))
        const = ctx.enter_context(tc.tile_pool(name="const", bufs=1))
        io = ctx.enter_context(tc.tile_pool(name="io", bufs=2))
        work = ctx.enter_context(tc.tile_pool(name="work", bufs=2))
        ppool = ctx.enter_context(tc.tile_pool(name="ppool", bufs=6))
        rpool = ctx.enter_context(tc.tile_pool(name="rpool", bufs=4))
        dram = ctx.enter_context(tc.tile_pool(name="dram", bufs=2, space="DRAM"))
        psum_agg = ctx.enter_context(
            tc.tile_pool(name="psum_agg", bufs=2, space="PSUM")
        )
        psum_tp = ctx.enter_context(tc.tile_pool(name="psum_tp", bufs=4, space="PSUM"))

        # ---- constants ----
        ident = const.tile([128, 128], fp32, name="ident")
        make_identity(nc, ident)

        bias_bc = const.tile([128, HF], fp32, name="bias_bc")
        nc.scalar.dma_start(out=bias_bc, in_=bias_d[:].partition_broadcast(128))

        w_sb = const.tile([128, DK, HF], fp32, name="w_sb")
        nc.scalar.dma_start(out=w_sb, in_=w_d[:].rearrange("(k p) m -> p k m", p=128))
        w_bf = const.tile([128, DK, HF], bf16, name="w_bf")
        nc.vector.tensor_copy(w_bf, w_sb)

        # W^T via PE transposes (to project a_src/a_dst back to input dim)
        wt_sb = const.tile([128, KK, d], fp32, name="wt_sb")
        for dk in range(DK):
            for kk in range(KK):
                tp = psum_tp.tile([128, 512], fp32, name="tp", tag="tp")
                nc.tensor.transpose(tp[:, 0:128], w_sb[:, dk, ts(kk, 128)], ident)
                nc.vector.tensor_copy(wt_sb[:, kk, ts(dk, 128)], tp[:, 0:128])

        # Block-diagonal attention vectors: A[hf, h'] = a_vec[h, f] iff h' == h
        a_tiles = {}
        for nm, src in (("asrc", asrc_d), ("adst", adst_d)):
            a_sb = const.tile([128, KK, h_heads], fp32, name=f"a_{nm}")
            nc.vector.memset(a_sb, 0.0)
            for hh in range(h_heads):
                kk = (hh * f_dim) // 128
                r0 = hh * f_dim - kk * 128
                nc.gpsimd.dma_start(out=a_sb[r0:r0 + f_dim, kk, hh], in_=src[hh, :])
            a_tiles[nm] = a_sb

        # w_vec[d, h] = sum_hf W^T[hf, d] * A[hf, h]  (so e = x @ w_vec)
        wv_bf = {}
        for nm in ("asrc", "adst"):
            wv_sb = const.tile([128, DK, h_heads], fp32, name=f"wv_{nm}")
            for dk in range(DK):
                tp = psum_tp.tile([128, 512], fp32, name="tp", tag="tp")
                for kk in range(KK):
                    nc.tensor.matmul(
                        tp[:, 0:h_heads],
                        wt_sb[:, kk, ts(dk, 128)],
                        a_tiles[nm][:, kk, :],
                        start=(kk == 0),
                        stop=(kk == KK - 1),
                    )
                nc.vector.tensor_copy(wv_sb[:, dk, :], tp[:, 0:h_heads])
            wvb = const.tile([128, DK, h_heads], bf16, name=f"wvb_{nm}")
            nc.vector.tensor_copy(wvb, wv_sb)
            wv_bf[nm] = wvb

        def pre(b):
            """Per-graph precompute: x^T, masks, features, e-derived tiles."""
            # adj loads first on the sync ring: no deps, stream immediately
            adj_tiles = []
            for it in range(NT):
                adj_sb = io.tile([128, n], fp32, name="adj_sb", tag="adj", bufs=4)
                nc.sync.dma_start(out=adj_sb, in_=adj_d[b][ts(it, 128), :])
                adj_tiles.append(adj_sb)

            # x loads directly as bf16 (SWDGE cast DMA)
            x_bf = io.tile([128, NT, d], bf16, name="x_bf", tag="xbf")
            nc.gpsimd.dma_start(
                out=x_bf, in_=x_d[b].rearrange("(t p) c -> p t c", p=128)
            )

            # x^T (bf16) via DMA xbar (contiguous [128, nt, dk, 128] dsts)
            xt_bf = io.tile([128, NT, DK, 128], bf16, name="xt_bf", tag="xt")
            for nt in range(NT):
                nc.sync.dma_start_transpose(xt_bf[:, nt, :, :], x_bf[:, nt, :])

            # mask compare + transposes
            m01 = io.tile([128, NT, NT, 128], bf16, name="m01", tag="m01")
            for it in range(NT):
                mnat = io.tile([128, n], bf16, name="mnat", tag="mnat")
                nc.vector.tensor_scalar(mnat, adj_tiles[it], 0.5, None, op0=OP.is_gt)
                nc.sync.dma_start_transpose(m01[:, it, :, :], mnat)

            # e rows first (they gate the wb broadcast + v-columns chain)
            e_sb = {}
            for nm in ("asrc", "adst"):
                esb = work.tile([h_heads, n], fp32, name="esb", tag=f"e_{nm}")
                for s, e in halves:
                    tpe = psum_tp.tile([128, 512], fp32, name="tpe", tag="tp")
                    nt0 = s // 128
                    for dk in range(DK):
                        nc.tensor.matmul(
                            tpe[0:h_heads, 0:e - s],
                            wv_bf[nm][:, dk, :],
                            xt_bf[:, nt0:nt0 + 4, dk, :],
                            start=(dk == 0),
                            stop=(dk == DK - 1),
                        )
                    nc.vector.tensor_copy(esb[:, s:e], tpe[0:h_heads, 0:e - s])
                e_sb[nm] = esb

            # v1 = exp(e_src), v2 = exp(0.2 e_src) (fp32 rows);
            # w = exp(-0.8 e_dst) (bf16 row for the broadcast)
            v1row = work.tile([h_heads, n], fp32, name="v1row", tag="v1r")
            nc.scalar.activation(v1row, e_sb["asrc"], AF.Exp)
            v2row = work.tile([h_heads, n], fp32, name="v2row", tag="v2r")
            nc.scalar.activation(v2row, e_sb["asrc"], AF.Exp, scale=0.2)
            wrow = work.tile([h_heads, n], bf16, name="wrow", tag="wr")
            nc.scalar.activation(wrow, e_sb["adst"], AF.Exp, scale=-0.8)

            # w rows -> DRAM -> per-head partition broadcast (bf16)
            wd = dram.tile([h_heads, n], bf16, name="wd", tag="wd")
            nc.scalar.dma_start(out=wd, in_=wrow)
            wb = io.tile([128, h_heads, n], bf16, name="wb", tag="wb")
            for hh in range(h_heads):
                nc.scalar.dma_start(
                    out=wb[:, hh, :], in_=wd[hh].partition_broadcast(128)
                )

            # v1/v2 -> per-partition columns via PE transpose: [128, NT, H]
            vcs = {}
            for vrow, tag in ((v1row, "v1c"), (v2row, "v2c")):
                vc = io.tile([128, NT, h_heads], fp32, name=f"c{tag}", tag=tag)
                for g0 in range(0, NT, 4):
                    tp = psum_tp.tile([128, 512], fp32, name="tp", tag="tp")
                    for q in range(4):
                        nc.tensor.transpose(
                            tp[:, q * h_heads:(q + 1) * h_heads],
                            vrow[:, ts(g0 + q, 128)],
                            ident[0:h_heads, 0:h_heads],
                        )
                    nc.vector.tensor_copy(
                        vc[:, g0:g0 + 4, :],
                        tp[:, 0:4 * h_heads].rearrange(
                            "p (t hh) -> p t hh", hh=h_heads
                        ),
                    )
                vcs[tag] = vc

            # features: h = x @ W -> [j, hh, ff] bf16 (+ ones col)
            ha = io.tile([128, NT, h_heads, F1], bf16, name="ha", tag="haug")
            nc.gpsimd.memset(ha[:, :, :, f_dim:F1], 1.0)
            for nt in range(NT):
                tp = psum_tp.tile([128, 512], fp32, name="tp", tag="tp")
                for dk in range(DK):
                    nc.tensor.matmul(
                        tp[:, 0:HF],
                        xt_bf[:, nt, dk, :],
                        w_bf[:, dk, :],
                        start=(dk == 0),
                        stop=(dk == DK - 1),
                    )
                nc.scalar.copy(
                    ha[:, nt, :, 0:f_dim],
                    tp[:, 0:HF].rearrange("p (hh ff) -> p hh ff", hh=h_heads),
                )

            ostage = io.tile([128, NT, HF], fp32, name="ostage", tag="ostage")
            return {
                "ha": ha, "m01": m01, "wb": wb,
                "v1c": vcs["v1c"], "v2c": vcs["v2c"], "ostage": ostage,
            }

        def head(b, t, hh):
            """One head: P tiles, masked, aggregated, normalized into ostage."""
            agg = psum_agg.tile([F1, n], fp32, name="agg", tag="agg")
            for jt in range(NT):
                # P = max(w[i]*v2[j], v1[j]); Pm = P * mask  (bf16)
                pm = ppool.tile([128, n], bf16, name="pm", tag="pm")
                nc.vector.tensor_scalar(
                    pm,
                    t["wb"][:, hh, :],
                    t["v2c"][:, jt, hh:hh + 1],
                    t["v1c"][:, jt, hh:hh + 1],
                    op0=OP.mult,
                    op1=OP.max,
                )
                eng = nc.gpsimd if (jt % 4 == 3) else nc.vector
                eng.tensor_tensor(pm, pm, t["m01"][:, :, jt, :], op=OP.mult)
                for s, e in halves:
                    nc.tensor.matmul(
                        agg[:, s:e],
                        t["ha"][:, jt, hh, :],
                        pm[:, s:e],
                        start=(jt == 0),
                        stop=(jt == NT - 1),
                    )

            # finalize: psum rows [0..F) = out^T, row F = denominator
            agg_sb = work.tile([F1, n], fp32, name="agg_sb", tag="aggsb")
            nc.scalar.copy(agg_sb, agg)
            for c in range(NT):
                tp = psum_tp.tile([128, 512], fp32, name="tp", tag="tp")
                nc.tensor.transpose(
                    tp[:, 0:F1], agg_sb[:, ts(c, 128)], ident[0:F1, 0:F1]
                )
                rcp = rpool.tile([128, 1], fp32, name="rcp", tag="rcp")
                nc.vector.reciprocal(rcp, tp[:, f_dim:F1])
                nc.scalar.activation(
                    t["ostage"][:, c, hh * f_dim:(hh + 1) * f_dim],
                    tp[:, 0:f_dim],
                    AF.Copy,
                    bias=0.0,
                    scale=rcp,
                )

        def finish(b, t):
            for c in range(NT):
                nc.vector.tensor_tensor(
                    t["ostage"][:, c, :], t["ostage"][:, c, :], bias_bc, op=OP.add
                )
            nc.scalar.dma_start(
                out=out_d[b].rearrange("(t p) m -> p t m", p=128), in_=t["ostage"]
            )

        # 2-stage software pipeline: pre(b+1) is emitted after the first head
        # of graph b so its DMA/PE work overlaps graph b's head compute.
        tiles = pre(0)
        for b in range(b_local):
            head(b, tiles, 0)
            nxt = pre(b + 1) if b + 1 < b_local else None
            for hh in range(1, h_heads):
                head(b, tiles, hh)
            finish(b, tiles)
            tiles = nxt

    nc.finalize()
    return nc


def _get_nc(shape_key):
    if shape_key not in _CACHE:
        _CACHE[shape_key] = _build(*shape_key)
    return _CACHE[shape_key]


def kernel(x, adj, W, a_src, a_dst, bias):
    from concourse.bass_utils import run_bass_kernel_spmd

    x = np.ascontiguousarray(x, dtype=np.float32)
    adj = np.ascontiguousarray(adj, dtype=np.float32)
    W = np.ascontiguousarray(W, dtype=np.float32)
    a_src = np.ascontiguousarray(a_src, dtype=np.float32)
    a_dst = np.ascontiguousarray(a_dst, dtype=np.float32)
    bias = np.ascontiguousarray(bias, dtype=np.float32)

    nc = _get_nc((B_LOCAL, N, D, H, F))
    in_maps = []
    for c in range(N_CORES):
        sl = slice(c * B_LOCAL, (c + 1) * B_LOCAL)
        in_maps.append(
            {
                "x": x[sl],
                "adj": adj[sl],
                "W": W,
                "a_src": a_src,
                "a_dst": a_dst,
                "bias": bias,
            }
        )
    res = run_bass_kernel_spmd(nc, in_maps, core_ids=list(range(N_CORES)))
    return np.concatenate([r["out"] for r in res.results], axis=0)
